# revision 1
# baseline (speedup 1.0000x reference)
"""Trainium2 Bass kernel: ContinuousLocationMap (scatter into per-batch coordinate maps).

Reference semantics (B=64 batches, N=4096 points each, bins 512x512):
  out[b, i, j, 0:2] = 0.63 background, 1.0 where a point landed
  out[b, i, j, 2]   = j/512 background, point x0 where it landed
  out[b, i, j, 3]   = i/512 background, point x1 where it landed
  with i = floor(x0*512), j = floor(x1*512); last write wins per cell.

Sharding: data parallel over batch, 8 cores x 8 batches, no collectives.

Per-core kernel:
  1. build one 4 MB background image [512, 512, 4] in SBUF (shared by all
     8 local batches), DMA it out 8x with big contiguous descriptors
  2. load the core's points [8, 4096, 2] laid out batch-major along the
     free dim, compute linearized cell indices (exact floor via
     int-roundtrip + compare, robust to HW rounding mode) and 16-byte
     records (1.0, 1.0, x0, x1) on DVE
  3. indirect (scatter) DMA writes the records over the background;
     HW descriptor unroll follows the SBUF-side AP: one descriptor (and
     one consumed index) per innermost contiguous run, so the source is
     shaped [128, recs, 4]. Each batch's scatter is ordered after that
     batch's background write only.
"""

import os

import numpy as np

from concourse import bacc, bass, mybir, tile
from concourse.bass_utils import run_bass_kernel_spmd
from concourse.tile_rust import add_dep_helper

B, N = 64, 4096
NCORES = 8
BL = B // NCORES  # local batches per core
BINS = 512
P = 128
WPB = N // P  # points per partition per batch (32)
MPP = BL * WPB  # points per partition total (256)
F32 = mybir.dt.float32
I32 = mybir.dt.int32

SCAT_MODE = os.environ.get("KERNEL_SCAT_MODE", "slim")  # slim | off
SMALL_OUTAP = os.environ.get("KERNEL_SMALL_OUTAP", "1") == "1"
REPS = int(os.environ.get("KERNEL_REPS", "1"))


def _build_body(tc, x_ap, y_ap, pool):
    nc = tc.nc
    # ---------------- background image ----------------
    # BG[p, t, j, c] = map row i = t*128 + p, col j, channel c
    BG = pool.tile([P, 4 * BINS * 4], F32)
    BG4 = BG[:].rearrange("p (t j c) -> p t j c", t=4, c=4)
    nc.vector.memset(BG4[:, :, :, 0:2], 0.63)

    JI = pool.tile([P, BINS], I32)
    nc.gpsimd.iota(JI[:], pattern=[[1, BINS]], base=0, channel_multiplier=0)
    JF = pool.tile([P, BINS], F32)
    nc.vector.tensor_copy(JF[:], JI[:])
    for t in range(4):
        nc.vector.tensor_scalar_mul(BG4[:, t, :, 2], JF[:], 1.0 / BINS)

    YI = pool.tile([P, 4], I32)
    nc.gpsimd.iota(YI[:], pattern=[[P, 4]], base=0, channel_multiplier=1)
    YF = pool.tile([P, 4], F32)
    nc.vector.tensor_copy(YF[:], YI[:])
    nc.vector.tensor_scalar_mul(YF[:], YF[:], 1.0 / BINS)
    for t in range(4):
        nc.vector.tensor_copy(BG4[:, t, :, 3], YF[:, t : t + 1].to_broadcast([P, BINS]))

    # ---------------- points -> indices + records ----------------
    # pts[p, b*64 + w] = x[b].flat[p*64 + w]: per batch, partition p holds
    # its points p*32 .. p*32+31 (x0,x1 interleaved)
    pts = pool.tile([P, MPP * 2], F32)
    x3d = x_ap.rearrange("b n c -> (b n c)").rearrange("(b2 p w) -> p b2 w", p=P, b2=BL)
    nc.sync.dma_start(out=pts[:].rearrange("p (b w) -> p b w", b=BL), in_=x3d)
    pts3 = pts[:].rearrange("p (m c) -> p m c", c=2)  # m = b*32 + w
    x0 = pts3[:, :, 0]
    x1 = pts3[:, :, 1]

    FL0 = pool.tile([P, MPP], F32)
    FL1 = pool.tile([P, MPP], F32)
    FL = [FL0, FL1]
    F = pool.tile([P, MPP], F32)
    G = pool.tile([P, MPP], F32)
    II = pool.tile([P, MPP], I32)
    for k, xs in enumerate((x0, x1)):
        nc.vector.tensor_scalar_mul(F[:], xs, float(BINS))
        nc.vector.tensor_copy(II[:], F[:])  # f32 -> i32, HW rounding unknown
        nc.vector.tensor_copy(FL[k][:], II[:])  # i32 -> f32, exact
        nc.vector.tensor_tensor(G[:], FL[k][:], F[:], op=mybir.AluOpType.is_gt)
        nc.vector.tensor_tensor(
            FL[k][:], FL[k][:], G[:], op=mybir.AluOpType.subtract
        )  # exact floor

    # cell = b*2^18 + i*512 + j, exact in f32 (< 2^21 < 2^24)
    CF = pool.tile([P, MPP], F32)
    nc.vector.scalar_tensor_tensor(
        CF[:],
        in0=FL[0][:],
        scalar=float(BINS),
        in1=FL[1][:],
        op0=mybir.AluOpType.mult,
        op1=mybir.AluOpType.add,
    )
    if not SMALL_OUTAP:
        BOFF = pool.tile([P, MPP], I32)
        nc.gpsimd.iota(
            BOFF[:], pattern=[[1, BL], [0, WPB]], base=0, channel_multiplier=0
        )
        BOFFF = pool.tile([P, MPP], F32)
        nc.vector.tensor_copy(BOFFF[:], BOFF[:])
        nc.vector.scalar_tensor_tensor(
            CF[:],
            in0=BOFFF[:],
            scalar=float(BINS * BINS),
            in1=CF[:],
            op0=mybir.AluOpType.mult,
            op1=mybir.AluOpType.add,
        )
    IDX = pool.tile([P, MPP], I32)
    nc.vector.tensor_copy(IDX[:], CF[:])

    REC = pool.tile([P, MPP * 4], F32)
    REC3 = REC[:].rearrange("p (m c) -> p m c", c=4)
    nc.vector.memset(REC3[:, :, 0:2], 1.0)
    nc.vector.tensor_copy(REC3[:, :, 2], x0)
    nc.vector.tensor_copy(REC3[:, :, 3], x1)

    # ---------------- DMA out ----------------
    ybg = y_ap.rearrange("b (t p) w c -> b p t (w c)", p=P)  # [BL,128,4,2048]
    BGtk = BG[:].rearrange("p (t k) -> p t k", t=4)
    bg_insts = []
    for b in range(BL):
        ins = nc.sync.dma_start(out=ybg[b], in_=BGtk)
        bg_insts.append(ins)

    last = bg_insts[-1]
    if SCAT_MODE == "off":
        return last
    if SMALL_OUTAP:
        # offset-0 view of ONE batch image; batch b reached via element_offset.
        # Cuts the dynamic out-AP row count 8x (per-call SWDGE cost scales
        # with it). NOTE: sim would see this as an OOB view write for b>0.
        yscat = y_ap[0].rearrange("h w c -> (h w) c")  # [262144, 4]
    else:
        yscat = y_ap.rearrange("b h w c -> (b h w) c")  # [2097152, 4], offset 0
    for b in range(BL):
        # one record per partition per call: in_ MUST be 2D [128, 4] (one
        # contiguous run per partition = one descriptor = one consumed index)
        calls = [
            (IDX[:, m : m + 1], REC[:, 4 * m : 4 * (m + 1)])
            for m in range(WPB * b, WPB * (b + 1))
        ]
        for idx_ap, rec_ap in calls:
            scat = nc.gpsimd.indirect_dma_start(
                out=yscat,
                out_offset=bass.IndirectOffsetOnAxis(ap=idx_ap, axis=0),
                in_=rec_ap,
                in_offset=None,
                element_offset=b * BINS * BINS * 4 if SMALL_OUTAP else 0,
            )
            add_dep_helper(
                scat.ins, bg_insts[b].ins, reason=f"scatter b{b} after background b{b}"
            )
            last = scat
    return last


def build_program(reps=REPS, timing=False):
    nc = bacc.Bacc("TRN2", target_bir_lowering=False, debug=False)
    x = nc.dram_tensor("batch", [BL, N, 2], F32, kind="ExternalInput")
    if timing:
        # timing variant: full work lands in internal DRAM scratch; tiny
        # external output avoids the 256 MB axon fetch that drowns wall-clock
        y = nc.dram_tensor("scratch", [BL, BINS, BINS, 4], F32)
        tout = nc.dram_tensor("out", [1, 4], F32, kind="ExternalOutput")
    else:
        y = nc.dram_tensor("out", [BL, BINS, BINS, 4], F32, kind="ExternalOutput")
    with tile.TileContext(nc) as tc:
        with tc.tile_pool(name="sbuf", bufs=1) as pool:
            prev_last = None
            for _ in range(reps):
                last = _build_body(tc, x.ap(), y.ap(), pool)
                if prev_last is not None:
                    add_dep_helper(last.ins, prev_last.ins, reason="rep chain")
                prev_last = last
            if timing:
                tt = pool.tile([1, 4], F32)
                nc.vector.memset(tt[:], 1.0)
                fin = nc.sync.dma_start(out=tout.ap(), in_=tt[:])
                add_dep_helper(fin.ins, prev_last.ins, reason="timing fence")
    nc.compile()
    return nc


_PROGRAM = None


def _get_program():
    global _PROGRAM
    if _PROGRAM is None:
        _PROGRAM = build_program()
    return _PROGRAM


def run_sharded(batch: np.ndarray, trace: bool = False, nc=None):
    """Run the SPMD kernel; returns (full_output, BassKernelResults)."""
    batch = np.ascontiguousarray(batch, dtype=np.float32)
    assert batch.shape == (B, N, 2), batch.shape
    if nc is None:
        nc = _get_program()
    in_maps = [{"batch": batch[c * BL : (c + 1) * BL]} for c in range(NCORES)]
    res = run_bass_kernel_spmd(nc, in_maps, list(range(NCORES)), trace=trace)
    out = np.concatenate([res.results[c]["out"] for c in range(NCORES)], axis=0)
    return out, res


def kernel(batch: np.ndarray) -> np.ndarray:
    out, _ = run_sharded(batch, trace=False)
    return out



# revision 14
# speedup vs baseline: 1.0899x; 1.0899x over previous
"""Trainium2 Bass kernel: ContinuousLocationMap (scatter into per-batch coordinate maps).

Reference semantics (B=64 batches, N=4096 points each, bins 512x512):
  out[b, i, j, 0:2] = 0.63 background, 1.0 where a point landed
  out[b, i, j, 2]   = j/512 background, point x0 where it landed
  out[b, i, j, 3]   = i/512 background, point x1 where it landed
  with i = floor(x0*512), j = floor(x1*512); last write wins per cell.

Sharding: data parallel over batch, 8 cores x 8 batches, no collectives.

Per-core kernel:
  1. build one 4 MB background image [512, 512, 4] in SBUF (shared by all
     8 local batches), DMA it out 8x with big contiguous descriptors
  2. load the core's points [8, 4096, 2] laid out batch-major along the
     free dim, compute linearized cell indices (exact floor via
     int-roundtrip + compare, robust to HW rounding mode) and 16-byte
     records (1.0, 1.0, x0, x1) on DVE
  3. indirect (scatter) DMA writes the records over the background.

HW facts for the scatter (established by on-device probes this session):
  * The walrus unroll of a dst-indirect InstDMACopy emits exactly ONE
    descriptor per PARTITION ROW of the index AP — 128 per call, hard cap.
    Multi-record sources ([128, K, 4] strided, [1, R, 4] packed) are
    mis-unrolled: descriptors walk the source linearly with the dim[-2]
    stride as both payload span and dst coefficient (observed 28-byte
    writes at idx*stride), producing garbage and (unbounded) wild DRAM
    writes that can wedge the device. So 32768 points/core = 256 calls
    of 128 records — the primitive's floor, ~1us SWDGE generation each,
    serialized on the Pool engine.
  * Tile's dependency tracker sees every background write and every
    scatter as "writes to y" and chained ALL of them pairwise on DMA
    COMPLETION semaphores — each scatter call stalled ~0.7us extra for
    its predecessor's full DMA round-trip (gen + 650ns DGE delay + 900ns
    sem propagation). Those edges are false sharing: all regions are
    disjoint except scatter-over-its-own-batch-background. Fix:
    dep_state.clear_tensor_accesses(y) after each emit, an explicit
    semaphore edge only background_b -> first scatter of batch b, and
    scheduler-only (sync=False) chaining for issue order everywhere else
    (the in-order Pool engine then provides ordering for free).
    CoreSim cost model: 241us -> 153us (bg-only floor 111us).
"""

import os

import numpy as np

from concourse import bacc, bass, mybir, tile
from concourse.bass_utils import run_bass_kernel_spmd
from concourse.tile_rust import add_dep_helper

B, N = 64, 4096
NCORES = 8
BL = B // NCORES  # local batches per core
BINS = 512
P = 128
WPB = N // P  # points per partition per batch (32)
MPP = BL * WPB  # points per partition total (256)
F32 = mybir.dt.float32
I32 = mybir.dt.int32

SCAT_MODE = os.environ.get("KERNEL_SCAT_MODE", "slim")  # slim | off
SMALL_OUTAP = os.environ.get("KERNEL_SMALL_OUTAP", "1") == "1"
REPS = int(os.environ.get("KERNEL_REPS", "1"))


def _build_body(tc, x_ap, y_ap, pool):
    nc = tc.nc
    # ---------------- background image ----------------
    # BG[p, t, j, c] = map row i = t*128 + p, col j, channel c
    BG = pool.tile([P, 4 * BINS * 4], F32)
    BG4 = BG[:].rearrange("p (t j c) -> p t j c", t=4, c=4)
    nc.vector.memset(BG4[:, :, :, 0:2], 0.63)

    JI = pool.tile([P, BINS], I32)
    nc.gpsimd.iota(JI[:], pattern=[[1, BINS]], base=0, channel_multiplier=0)
    JF = pool.tile([P, BINS], F32)
    nc.vector.tensor_copy(JF[:], JI[:])
    for t in range(4):
        nc.vector.tensor_scalar_mul(BG4[:, t, :, 2], JF[:], 1.0 / BINS)

    YI = pool.tile([P, 4], I32)
    nc.gpsimd.iota(YI[:], pattern=[[P, 4]], base=0, channel_multiplier=1)
    YF = pool.tile([P, 4], F32)
    nc.vector.tensor_copy(YF[:], YI[:])
    nc.vector.tensor_scalar_mul(YF[:], YF[:], 1.0 / BINS)
    for t in range(4):
        nc.vector.tensor_copy(BG4[:, t, :, 3], YF[:, t : t + 1].to_broadcast([P, BINS]))

    # ---------------- points -> indices + records ----------------
    # pts[p, b*64 + w] = x[b].flat[p*64 + w]: per batch, partition p holds
    # its points p*32 .. p*32+31 (x0,x1 interleaved)
    pts = pool.tile([P, MPP * 2], F32)
    x3d = x_ap.rearrange("b n c -> (b n c)").rearrange("(b2 p w) -> p b2 w", p=P, b2=BL)
    nc.sync.dma_start(out=pts[:].rearrange("p (b w) -> p b w", b=BL), in_=x3d)
    pts3 = pts[:].rearrange("p (m c) -> p m c", c=2)  # m = b*32 + w
    x0 = pts3[:, :, 0]
    x1 = pts3[:, :, 1]

    FL0 = pool.tile([P, MPP], F32)
    FL1 = pool.tile([P, MPP], F32)
    FL = [FL0, FL1]
    F = pool.tile([P, MPP], F32)
    G = pool.tile([P, MPP], F32)
    II = pool.tile([P, MPP], I32)
    for k, xs in enumerate((x0, x1)):
        nc.vector.tensor_scalar_mul(F[:], xs, float(BINS))
        nc.vector.tensor_copy(II[:], F[:])  # f32 -> i32, HW rounding unknown
        nc.vector.tensor_copy(FL[k][:], II[:])  # i32 -> f32, exact
        nc.vector.tensor_tensor(G[:], FL[k][:], F[:], op=mybir.AluOpType.is_gt)
        nc.vector.tensor_tensor(
            FL[k][:], FL[k][:], G[:], op=mybir.AluOpType.subtract
        )  # exact floor

    # cell = b*2^18 + i*512 + j, exact in f32 (< 2^21 < 2^24)
    CF = pool.tile([P, MPP], F32)
    nc.vector.scalar_tensor_tensor(
        CF[:],
        in0=FL[0][:],
        scalar=float(BINS),
        in1=FL[1][:],
        op0=mybir.AluOpType.mult,
        op1=mybir.AluOpType.add,
    )
    if not SMALL_OUTAP:
        BOFF = pool.tile([P, MPP], I32)
        nc.gpsimd.iota(
            BOFF[:], pattern=[[1, BL], [0, WPB]], base=0, channel_multiplier=0
        )
        BOFFF = pool.tile([P, MPP], F32)
        nc.vector.tensor_copy(BOFFF[:], BOFF[:])
        nc.vector.scalar_tensor_tensor(
            CF[:],
            in0=BOFFF[:],
            scalar=float(BINS * BINS),
            in1=CF[:],
            op0=mybir.AluOpType.mult,
            op1=mybir.AluOpType.add,
        )
    IDX = pool.tile([P, MPP], I32)
    nc.vector.tensor_copy(IDX[:], CF[:])

    REC = pool.tile([P, MPP * 4], F32)
    REC3 = REC[:].rearrange("p (m c) -> p m c", c=4)
    nc.vector.memset(REC3[:, :, 0:2], 1.0)
    nc.vector.tensor_copy(REC3[:, :, 2], x0)
    nc.vector.tensor_copy(REC3[:, :, 3], x1)

    # ---------------- DMA out ----------------
    # All writes into y (backgrounds of different batches; scatters vs the
    # backgrounds of other batches) are disjoint except scatter-over-its-own-
    # batch-background, which is ordered explicitly below. Tile's dependency
    # tracker only sees "same tensor, same view" and would chain EVERY pair
    # of these DMAs on completion semaphores — on HW that serialized all 256
    # scatter calls on full DMA round-trips (~1.65us each). Clearing the
    # tracked accesses of y after each emit suppresses those false WAW edges.
    yname = y_ap.tensor.name

    def clear_y():
        tc.dep_state.clear_tensor_accesses(yname)

    ybg = y_ap.rearrange("b (t p) w c -> b p t (w c)", p=P)  # [BL,128,4,2048]
    BGtk = BG[:].rearrange("p (t k) -> p t k", t=4)
    bg_insts = []
    for b in range(BL):
        ins = nc.sync.dma_start(out=ybg[b], in_=BGtk)
        clear_y()
        if b > 0:
            # keep issue order without a semaphore (scheduler-only edge)
            add_dep_helper(ins.ins, bg_insts[b - 1].ins, sync=False, reason="bg order")
        bg_insts.append(ins)

    last = bg_insts[-1]
    if SCAT_MODE == "off":
        return last
    if SMALL_OUTAP:
        # offset-0 view of ONE batch image; batch b reached via element_offset.
        # Cuts the dynamic out-AP row count 8x (per-call SWDGE cost scales
        # with it). NOTE: sim would see this as an OOB view write for b>0.
        yscat = y_ap[0].rearrange("h w c -> (h w) c")  # [262144, 4]
    else:
        yscat = y_ap.rearrange("b h w c -> (b h w) c")  # [2097152, 4], offset 0
    for b in range(BL):
        # one record per partition per call: in_ MUST be 2D [128, 4]. The HW
        # unroll emits exactly one descriptor per idx-AP partition row (128
        # max per call) — larger idx APs or 3-dim sources generate garbage
        # descriptors, so 32 calls per batch is the primitive's floor.
        for k, m in enumerate(range(WPB * b, WPB * (b + 1))):
            scat = nc.gpsimd.indirect_dma_start(
                out=yscat,
                out_offset=bass.IndirectOffsetOnAxis(ap=IDX[:, m : m + 1], axis=0),
                in_=REC[:, 4 * m : 4 * (m + 1)],
                in_offset=None,
                element_offset=b * BINS * BINS * 4 if SMALL_OUTAP else 0,
            )
            clear_y()
            if k == 0:
                # Only the first scatter of the batch waits (with a real
                # semaphore) on the batch's background write; the rest are
                # chained to it with scheduler-only no_sync edges, so the
                # in-order Pool engine provides the ordering for free
                # instead of 31 more completion-semaphore stalls per batch.
                add_dep_helper(
                    scat.ins,
                    bg_insts[b].ins,
                    reason=f"scatter b{b} after background b{b}",
                )
            if last is not None:
                add_dep_helper(scat.ins, last.ins, sync=False, reason="scat order")
            last = scat
    return last


def build_program(reps=REPS, timing=False):
    nc = bacc.Bacc("TRN2", target_bir_lowering=False, debug=False)
    x = nc.dram_tensor("batch", [BL, N, 2], F32, kind="ExternalInput")
    if timing:
        # timing variant: full work lands in internal DRAM scratch; tiny
        # external output avoids the 256 MB axon fetch that drowns wall-clock
        y = nc.dram_tensor("scratch", [BL, BINS, BINS, 4], F32)
        tout = nc.dram_tensor("out", [1, 4], F32, kind="ExternalOutput")
    else:
        y = nc.dram_tensor("out", [BL, BINS, BINS, 4], F32, kind="ExternalOutput")
    with tile.TileContext(nc) as tc:
        with tc.tile_pool(name="sbuf", bufs=1) as pool:
            prev_last = None
            for _ in range(reps):
                last = _build_body(tc, x.ap(), y.ap(), pool)
                if prev_last is not None:
                    add_dep_helper(last.ins, prev_last.ins, reason="rep chain")
                prev_last = last
            if timing:
                tt = pool.tile([1, 4], F32)
                nc.vector.memset(tt[:], 1.0)
                fin = nc.sync.dma_start(out=tout.ap(), in_=tt[:])
                add_dep_helper(fin.ins, prev_last.ins, reason="timing fence")
    nc.compile()
    return nc


_PROGRAM = None


def _get_program():
    global _PROGRAM
    if _PROGRAM is None:
        _PROGRAM = build_program()
    return _PROGRAM


def run_sharded(batch: np.ndarray, trace: bool = False, nc=None):
    """Run the SPMD kernel; returns (full_output, BassKernelResults)."""
    batch = np.ascontiguousarray(batch, dtype=np.float32)
    assert batch.shape == (B, N, 2), batch.shape
    if nc is None:
        nc = _get_program()
    in_maps = [{"batch": batch[c * BL : (c + 1) * BL]} for c in range(NCORES)]
    res = run_bass_kernel_spmd(nc, in_maps, list(range(NCORES)), trace=trace)
    out = np.concatenate([res.results[c]["out"] for c in range(NCORES)], axis=0)
    return out, res


def kernel(batch: np.ndarray) -> np.ndarray:
    out, _ = run_sharded(batch, trace=False)
    return out

